# revision 1
# baseline (speedup 1.0000x reference)
"""Blockwise-int4-dequant GEMM (BlkQ4Linear) for 8 Trainium2 NeuronCores.

Problem: out[m, n] = sum_k input[m, k] * w[n, k] + bias[n],
         w = (q_weights - q_zp) * q_scales   (block size 64 along K)
         M, K, N = 4096, 4096, 11008

Strategy (column-parallel over out_features, fp8 DoubleRow matmuls):
  - Shard w/bias along N across 8 cores (Nc = 1376 each); replicate the
    input; no collectives -- host concatenates per-core [M, Nc] outputs.
  - Host-side prep: dequantize w in fp32, then build an fp8e4m3
    multi-term representation:
        x_hi = e4m3(x),  x_lo = e4m3(x - x_hi)          [all k]
        w_hi = e4m3(w),  w_lo = e4m3(w - w_hi)          [first PHI k-tiles]
    The device computes  x_hi*w_hi (all 32 k-tiles) + x_lo*w_hi (first
    30 k-tiles) + x_hi*w_lo (first PHI=22 k-tiles) per output tile, all
    as fp8e4m3 DoubleRow matmuls (2 contraction k-tiles per instruction
    at 0.5 cycles/row), accumulating in fp32 PSUM.  Measured end-to-end
    rel err of this representation on the reference inputs: 1.80e-2
    (gate: 2e-2).
  - Per core: keep w_hi [K, Nc] and w_lo [PHI*128, Nc] resident in SBUF
    (fp8), stream 256-row x slabs (hi+lo), add bias during PSUM->SBUF
    eviction on DVE, DMA f32 output tiles.
"""
import sys

for _p in ("/opt/trn_rl_repo", "/root/.axon_site/_ro/trn_rl_repo"):
    if _p not in sys.path:
        sys.path.insert(0, _p)

import numpy as np
import ml_dtypes

import concourse.bacc as bacc
import concourse.tile as tile
from concourse import mybir
from concourse.bass_utils import run_bass_kernel_spmd

BF16 = mybir.dt.bfloat16
F32 = mybir.dt.float32
FP8 = mybir.dt.float8e4
DR = mybir.MatmulPerfMode.DoubleRow
BLOCK = 64

M, K, N = 4096, 4096, 11008
NCORES = 8
NC_SHARD = N // NCORES  # 1376
MGW = 256  # m-group width (x slab)
PHI = 22   # k-tiles (of 32) receiving the w_lo correction term


def _n_slices(nc_width, cap=512):
    out, o = [], 0
    while o < nc_width:
        w = min(cap, nc_width - o)
        out.append((o, w))
        o += w
    return out


def build_program(M, K, Nc, MGW=256):
    """Build + compile the per-core Bass program (identical on all cores)."""
    assert K % 256 == 0 and M % MGW == 0 and MGW % 128 == 0
    KT = K // 128          # 32 k-tiles
    NP = KT // 2           # 16 DoubleRow k-tile pairs
    PP = PHI // 2          # 11 w-corr pairs
    NMI = MGW // 128

    nc = bacc.Bacc("TRN2", target_bir_lowering=False, debug=False)

    # x in mi-major layout: x*[mg, mi, p, kt*128+j] = x[mg*MGW+mi*128+j,
    # kt*128+p] -- per-m-tile halves are contiguous so the startup stream
    # can fetch only the half the merged mg1-head actually consumes
    xh = nc.dram_tensor("xh", [M // MGW, NMI, 128, KT * 128], FP8,
                        kind="ExternalInput")
    xl = nc.dram_tensor("xl", [M // MGW, NMI, 128, KT * 128], FP8,
                        kind="ExternalInput")
    wh = nc.dram_tensor("wh", [K, Nc], FP8, kind="ExternalInput")
    wl = nc.dram_tensor("wl", [PHI * 128, Nc], FP8, kind="ExternalInput")
    ob = nc.dram_tensor("ob", [1, Nc], F32, kind="ExternalInput")
    out = nc.dram_tensor("out", [M, Nc], F32, kind="ExternalOutput")

    NS = _n_slices(Nc)

    # W stream group sizes (in k-tiles, always even so DR pairs never
    # straddle a group): small first groups so the first pair is ready fast.
    def groups(total):
        gs, rem = [], total
        for s in (2, 2):
            if rem:
                gs.append(min(s, rem))
                rem -= gs[-1]
        while rem:
            gs.append(min(4, rem))
            rem -= gs[-1]
        return gs

    WH_G = groups(KT)
    WL_G = groups(PHI)

    with tile.TileContext(nc) as tc:
        with (
            tc.tile_pool(name="wres", bufs=1) as wres_pool,
            tc.tile_pool(name="const", bufs=1) as const_pool,
            tc.tile_pool(name="xs", bufs=8) as xs_pool,
            tc.tile_pool(name="psum", bufs=8, space="PSUM") as psum_pool,
            tc.tile_pool(name="osb", bufs=3) as osb_pool,
            tc.tile_pool(name="parts", bufs=2) as parts_pool,
        ):
            # --- startup DMA stream, one queue (sync), hand-woven order ---
            # bias -> mg0 x chunks woven between w_hi groups -> mg1 x ->
            # w_lo -> mg2 x.  PE consumption of mg0 (main+xcorr interleaved)
            # paces the w_hi arrival; w_lo is only needed from mg1's
            # deferred w-corr phase onwards.
            bias_rep = const_pool.tile([128, Nc], F32)
            nc.scalar.dma_start(
                bias_rep[:].unsqueeze(1),
                ob[0:1, :].partition_broadcast(128),
            )

            wht = wres_pool.tile([128, KT, Nc], FP8, name="wht")
            wlt = wres_pool.tile([128, PHI, Nc], FP8, name="wlt")

            def w_group(dst, src, g0, gs):
                nc.sync.dma_start(
                    dst[:, g0 : g0 + gs, :],
                    src[g0 * 128 : (g0 + gs) * 128, :].rearrange(
                        "(j p) n -> p j n", p=128
                    ),
                )

            def slab_tile():
                return xs_pool.tile([128, NMI, KT, 128], FP8, name="xs")

            def slab_chunk(t, src, mg, mi, ci, nchunk):
                cs = KT // nchunk
                nc.sync.dma_start(
                    t[:, mi, ci * cs : (ci + 1) * cs, :],
                    src[mg, mi].rearrange("p (kt j) -> p kt j", kt=KT)[
                        :, ci * cs : (ci + 1) * cs, :
                    ],
                )

            pre = {}
            for mg in range(3):
                pre[mg] = (slab_tile(), slab_tile())

            # weave: x-mg0 and x-mg1 chunks between the w_hi groups
            wsched = {
                0: [(0, 0, 0, 0), (0, 0, 1, 0)],   # x0h mi0/mi1 c0
                1: [(0, 1, 0, 0), (0, 1, 1, 0)],   # x0l mi0/mi1 c0
                2: [(1, 0, 0, 0), (1, 1, 0, 0)],   # x1h/x1l mi0 c0
                3: [(0, 0, 0, 1), (0, 0, 1, 1)],   # x0h mi0/mi1 c1
                4: [(0, 1, 0, 1), (0, 1, 1, 1)],   # x0l mi0/mi1 c1
                5: [(1, 0, 0, 1), (1, 1, 0, 1)],   # x1h/x1l mi0 c1
            }
            wg = 0
            for gi, gs in enumerate(WH_G):
                w_group(wht, wh, wg, gs)
                wg += gs
                for mgsel, part, mi, ci in wsched.get(gi, []):
                    slab_chunk(pre[mgsel][part], (xh, xl)[part],
                               mgsel, mi, ci, 2)
            for part in (0, 1):
                for ci in (0, 1):
                    slab_chunk(pre[1][part], (xh, xl)[part], 1, 1, ci, 2)
            wg = 0
            for gs in WL_G:
                w_group(wlt, wl, wg, gs)
                wg += gs
            for mi in range(NMI):
                slab_chunk(pre[2][0], xh, 2, mi, 0, 1)
                slab_chunk(pre[2][1], xl, 2, mi, 0, 1)

            def do_mm(psums_mi, mi, xt, wt, t, start, stop):
                lhsT = xt[:, mi, 2 * t : 2 * t + 2, :]
                for si, (n0, nw) in enumerate(NS):
                    nc.tensor.matmul(
                        psums_mi[si][:, :nw],
                        lhsT,
                        wt[:, 2 * t : 2 * t + 2, n0 : n0 + nw],
                        start=start,
                        stop=stop,
                        perf_mode=DR,
                    )

            def evict(psums_mi, mi, mg, add_from=None):
                ot = osb_pool.tile([128, Nc], F32, name="ot")
                base = bias_rep if add_from is None else add_from
                for si, (n0, nw) in enumerate(NS):
                    nc.vector.tensor_add(
                        ot[:, n0 : n0 + nw],
                        psums_mi[si][:, :nw],
                        base[:, n0 : n0 + nw],
                    )
                m0 = mg * MGW + mi * 128
                nc.scalar.dma_start(out[m0 : m0 + 128, :], ot[:])

            # --- mg0 pass 1 + head of mg1: interleaved per k-pair.
            # mg0 (6 psum banks) plus mg1.mi0's first two slices (the 2
            # remaining banks), so the x1 bytes streaming in behind w_hi
            # buy PE work during the startup window.
            psums0 = [
                [psum_pool.tile([128, 512], F32, name="pt") for _ in NS]
                for _ in range(NMI)
            ]
            xsh0, xsl0 = pre[0]
            xsh1, xsl1 = pre[1]
            ps1_head = [psum_pool.tile([128, 512], F32, name="pt")
                        for _ in range(2)]

            def mg1_head_mm(xt, t):
                for si in (0, 1):
                    n0, nw = NS[si]
                    nc.tensor.matmul(
                        ps1_head[si][:, :nw],
                        xt[:, 0, 2*t:2*t+2, :],
                        wht[:, 2*t:2*t+2, n0:n0+nw],
                        start=(t == 0 and xt is xsh1),
                        stop=False, perf_mode=DR)

            for t in range(NP):
                for mi in range(NMI):
                    do_mm(psums0[mi], mi, xsh0, wht, t,
                          start=(t == 0), stop=(t == NP - 1))
                mg1_head_mm(xsh1, t)
                if t < NP - 1:
                    for mi in range(NMI):
                        do_mm(psums0[mi], mi, xsl0, wht, t,
                              start=False, stop=False)
                    mg1_head_mm(xsl1, t)
            parts = []
            for mi in range(NMI):
                pt = parts_pool.tile([128, Nc], F32, name="part")
                for si, (n0, nw) in enumerate(NS):
                    nc.vector.tensor_add(
                        pt[:, n0 : n0 + nw],
                        psums0[mi][si][:, :nw],
                        bias_rep[:, n0 : n0 + nw],
                    )
                parts.append(pt)

            # --- mg1 continuation: remaining main+x-corr chains, then
            # w-corr for all slices; s0/s1 of mi0 already accumulated ---
            psums1 = [[None] * len(NS) for _ in range(NMI)]
            psums1[0][0], psums1[0][1] = ps1_head
            for mi in range(NMI):
                for si, (n0, nw) in enumerate(NS):
                    if mi == 0 and si < 2:
                        continue
                    ps = psum_pool.tile([128, 512], F32, name="pt")
                    psums1[mi][si] = ps
                    for t in range(NP):
                        nc.tensor.matmul(
                            ps[:, :nw],
                            xsh1[:, mi, 2*t:2*t+2, :],
                            wht[:, 2*t:2*t+2, n0:n0+nw],
                            start=(t == 0), stop=False, perf_mode=DR)
                        if t < NP - 1:
                            nc.tensor.matmul(
                                ps[:, :nw],
                                xsl1[:, mi, 2*t:2*t+2, :],
                                wht[:, 2*t:2*t+2, n0:n0+nw],
                                start=False, stop=False, perf_mode=DR)
            for mi in range(NMI):
                for t in range(PP):
                    do_mm(psums1[mi], mi, xsh1, wlt, t,
                          start=False, stop=(t == PP - 1))
                evict(psums1[mi], mi, 1)

            # --- mg0 pass 2: w-corr + final eviction ---
            for mi in range(NMI):
                psums_mi = [
                    psum_pool.tile([128, 512], F32, name="pt") for _ in NS
                ]
                for t in range(PP):
                    do_mm(psums_mi, mi, xsh0, wlt, t,
                          start=(t == 0), stop=(t == PP - 1))
                evict(psums_mi, mi, 0, add_from=parts[mi])

            # --- mg2..: steady state ---
            for mg in range(2, M // MGW):
                if mg in pre:
                    xsh_g, xsl_g = pre[mg]
                else:
                    xsh_g, xsl_g = slab_tile(), slab_tile()
                    for mi in range(NMI):
                        slab_chunk(xsh_g, xh, mg, mi, 0, 1)
                        slab_chunk(xsl_g, xl, mg, mi, 0, 1)
                for mi in range(NMI):
                    last = mg == M // MGW - 1 and mi == NMI - 1
                    psums_mi = [
                        psum_pool.tile([128, 512], F32, name="pt")
                        for _ in NS
                    ]
                    if not last:
                        for t in range(NP):
                            do_mm(psums_mi, mi, xsh_g, wht, t,
                                  start=(t == 0), stop=False)
                        for t in range(NP - 1):
                            do_mm(psums_mi, mi, xsl_g, wht, t,
                                  start=False, stop=False)
                        for t in range(PP):
                            do_mm(psums_mi, mi, xsh_g, wlt, t,
                                  start=False, stop=(t == PP - 1))
                        evict(psums_mi, mi, mg)
                    else:
                        # drain the last m-tile slice-by-slice; the final
                        # (narrow) slice is additionally split along k into
                        # two psum banks so the closing eviction+DMA only
                        # trails half a chain
                        ot = osb_pool.tile([128, Nc], F32, name="ot")
                        m0 = mg * MGW + mi * 128
                        terms_list = (
                            [("m", t) for t in range(NP)]
                            + [("x", t) for t in range(NP - 1)]
                            + [("w", t) for t in range(PP)]
                        )

                        def emit_terms(ps, nw, n0, tl, first, last):
                            for i, (kind, t) in enumerate(tl):
                                xt = xsl_g if kind == "x" else xsh_g
                                wt = wlt if kind == "w" else wht
                                nc.tensor.matmul(
                                    ps[:, :nw],
                                    xt[:, mi, 2*t:2*t+2, :],
                                    wt[:, 2*t:2*t+2, n0:n0+nw],
                                    start=first and i == 0,
                                    stop=last and i == len(tl) - 1,
                                    perf_mode=DR)

                        for si, (n0, nw) in enumerate(NS[:-1]):
                            ps = psums_mi[si]
                            emit_terms(ps, nw, n0, terms_list, True, True)
                            nc.vector.tensor_add(
                                ot[:, n0:n0+nw], ps[:, :nw],
                                bias_rep[:, n0:n0+nw])
                            nc.scalar.dma_start(
                                out[m0:m0+128, n0:n0+nw],
                                ot[:, n0:n0+nw])
                        n0, nw = NS[-1]
                        half = len(terms_list) // 2
                        psA = psums_mi[len(NS) - 1]
                        psB = psum_pool.tile([128, 512], F32, name="pt")
                        emit_terms(psA, nw, n0, terms_list[:half], True, True)
                        tA = osb_pool.tile([128, 512], F32, name="tA")
                        nc.vector.tensor_add(
                            tA[:, :nw], psA[:, :nw], bias_rep[:, n0:n0+nw])
                        emit_terms(psB, nw, n0, terms_list[half:], True, True)
                        nc.vector.tensor_add(
                            ot[:, n0:n0+nw], psB[:, :nw], tA[:, :nw])
                        nc.scalar.dma_start(
                            out[m0:m0+128, n0:n0+nw], ot[:, n0:n0+nw])

    nc.compile()
    return nc


def _e4m3_split(a):
    """Return (hi, lo) e4m3 arrays with hi + lo ~= a (both as fp8 views)."""
    f8 = ml_dtypes.float8_e4m3
    hi = a.astype(f8)
    lo = (a - hi.astype(np.float32)).astype(f8)
    return hi, lo


def make_in_maps(input_np, q_weights, q_scales, q_zp, bias, ncores=NCORES):
    """Host-side prep: dequant / fp8-split / transpose / shard."""
    n = q_weights.shape[0]
    nc_sh = n // ncores
    m, k = input_np.shape
    kt = k // 128

    xhi, xlo = _e4m3_split(input_np.astype(np.float32))

    nmi = MGW // 128

    def xfmt(a):
        # [mg, mi, p, kt, j] = a[mg*MGW+mi*128+j, kt*128+p]
        return np.ascontiguousarray(
            a.T.reshape(kt, 128, m // MGW, nmi, 128).transpose(2, 3, 1, 0, 4)
        ).reshape(m // MGW, nmi, 128, kt * 128)

    xh = xfmt(xhi)
    xl = xfmt(xlo)

    maps = []
    for i in range(ncores):
        sl = slice(i * nc_sh, (i + 1) * nc_sh)
        w = (
            (q_weights[sl] - np.repeat(q_zp[sl], BLOCK, axis=1))
            .astype(np.float32)
            * np.repeat(q_scales[sl], BLOCK, axis=1)
        )
        whi, wlo = _e4m3_split(w)
        maps.append(
            {
                "xh": xh,
                "xl": xl,
                "wh": np.ascontiguousarray(whi.T),
                "wl": np.ascontiguousarray(wlo.T[: PHI * 128, :]),
                "ob": bias[sl].astype(np.float32).reshape(1, nc_sh),
            }
        )
    return maps


_PROGRAM = None


def _get_program():
    global _PROGRAM
    if _PROGRAM is None:
        _PROGRAM = build_program(M, K, NC_SHARD, MGW)
    return _PROGRAM


def kernel(input, q_weights, q_scales, q_zp, bias):
    """Full unsharded inputs -> full [M, N] float32 output."""
    input = np.asarray(input, dtype=np.float32)
    q_weights = np.asarray(q_weights, dtype=np.int32)
    q_scales = np.asarray(q_scales, dtype=np.float32)
    q_zp = np.asarray(q_zp, dtype=np.int32)
    bias = np.asarray(bias, dtype=np.float32)

    nc = _get_program()
    maps = make_in_maps(input, q_weights, q_scales, q_zp, bias)
    res = run_bass_kernel_spmd(nc, maps, core_ids=list(range(NCORES)))
    return np.concatenate(
        [res.results[i]["out"] for i in range(NCORES)], axis=1
    )



# revision 2
# speedup vs baseline: 1.1522x; 1.1522x over previous
"""Blockwise-int4-dequant GEMM (BlkQ4Linear) for 8 Trainium2 NeuronCores.

Problem: out[m, n] = sum_k input[m, k] * w[n, k] + bias[n],
         w = (q_weights - q_zp) * q_scales   (block size 64 along K)
         M, K, N = 4096, 4096, 11008

Strategy (column-parallel over out_features, fp8 DoubleRow matmuls with a
block-mean-recentered weight representation):
  - Shard w/bias along N across 8 cores (Nc = 1376 each); replicate the
    input; no collectives -- host concatenates per-core [M, Nc] outputs.
  - Weight recentering: w = s*(q - c) + s*(c - zp) with c = per-(n,block)
    mean of q.  The first part w' = s*(q-c) has ~2x less power than the
    zp-centered w, so e4m3 rounding noise on it contributes ~2x less
    output error; the second part is constant per 64-block along k, so
      sum_k x[m,k]*s[n,b]*(c-zp)[n,b] = sum_b xs[m,b] * G[n,b]
    with xs = per-block sums of x -- a rank-64 bf16 GEMM on device
    (contraction 65 with a ones-row that also carries the bias).
  - fp8 split: x = xh + xl (e4m3 hi/lo, both full K), w' = wh + wl with
    wl kept only on the first WLT=4 k-tiles.  Device computes
      xh*wh (16 DR pairs) + xl*wh (16) + xh*wl (2) + [xs|1]*[G|bias] (bf16)
    per output m-tile, accumulating in fp32 PSUM.  Measured end-to-end
    rel err on the reference inputs: 1.94e-2 (gate: 2e-2).
  - Per core: keep wh [K, Nc], wl [WLT*128, Nc] (fp8), G^T [65, Nc] and
    xs^T [65, M] (bf16) resident in SBUF, stream 256-row x slabs (hi+lo),
    evict PSUM via DVE copy, DMA f32 output tiles.
"""
import sys

for _p in ("/opt/trn_rl_repo", "/root/.axon_site/_ro/trn_rl_repo"):
    if _p not in sys.path:
        sys.path.insert(0, _p)

import numpy as np
import ml_dtypes

import concourse.bacc as bacc
import concourse.tile as tile
from concourse import mybir
from concourse.bass_utils import run_bass_kernel_spmd

BF16 = mybir.dt.bfloat16
F32 = mybir.dt.float32
FP8 = mybir.dt.float8e4
DR = mybir.MatmulPerfMode.DoubleRow
BLOCK = 64

M, K, N = 4096, 4096, 11008
NCORES = 8
NC_SHARD = N // NCORES  # 1376
MGW = 256  # m-group width (x slab)
WLT = 4    # k-tiles (of 32) receiving the w_lo correction term
NB = K // BLOCK  # 64 scale blocks


def _n_slices(nc_width, cap=512):
    out, o = [], 0
    while o < nc_width:
        w = min(cap, nc_width - o)
        out.append((o, w))
        o += w
    return out


def build_program(M, K, Nc, MGW=256):
    """Build + compile the per-core Bass program (identical on all cores)."""
    assert K % 256 == 0 and M % MGW == 0 and MGW % 128 == 0
    KT = K // 128          # 32 k-tiles
    NP = KT // 2           # 16 DoubleRow k-tile pairs
    WLP = WLT // 2         # 2 w-corr pairs
    NMI = MGW // 128

    nc = bacc.Bacc("TRN2", target_bir_lowering=False, debug=False)

    # x in mi-major layout: x*[mg, mi, p, kt*128+j] = x[mg*MGW+mi*128+j,
    # kt*128+p] -- per-m-tile halves are contiguous so the startup stream
    # can fetch only the half the early chains actually consume
    xh = nc.dram_tensor("xh", [M // MGW, NMI, 128, KT * 128], FP8,
                        kind="ExternalInput")
    xl = nc.dram_tensor("xl", [M // MGW, NMI, 128, KT * 128], FP8,
                        kind="ExternalInput")
    wh = nc.dram_tensor("wh", [K, Nc], FP8, kind="ExternalInput")
    wl = nc.dram_tensor("wl", [WLT * 128, Nc], FP8, kind="ExternalInput")
    # correction GEMM operands: contraction 65 = 64 block-sums + ones row
    gx = nc.dram_tensor("gx", [65, M], BF16, kind="ExternalInput")
    gw = nc.dram_tensor("gw", [65, Nc], BF16, kind="ExternalInput")
    out = nc.dram_tensor("out", [M, Nc], F32, kind="ExternalOutput")

    NS = _n_slices(Nc)

    # W stream group sizes (in k-tiles, always even so DR pairs never
    # straddle a group): small first groups so the first pair is ready fast.
    def groups(total):
        gs, rem = [], total
        for s in (2, 2):
            if rem:
                gs.append(min(s, rem))
                rem -= gs[-1]
        while rem:
            gs.append(min(4, rem))
            rem -= gs[-1]
        return gs

    WH_G = groups(KT)

    with tile.TileContext(nc) as tc:
        with (
            tc.tile_pool(name="wres", bufs=1) as wres_pool,
            tc.tile_pool(name="xs", bufs=8) as xs_pool,
            tc.tile_pool(name="psum", bufs=8, space="PSUM") as psum_pool,
            tc.tile_pool(name="osb", bufs=3) as osb_pool,
        ):
            wht = wres_pool.tile([128, KT, Nc], FP8, name="wht")
            wlt = wres_pool.tile([128, WLT, Nc], FP8, name="wlt")
            gxt = wres_pool.tile([65, M], BF16, name="gxt")
            gwt = wres_pool.tile([65, Nc], BF16, name="gwt")

            def w_group(dst, src, g0, gs):
                nc.sync.dma_start(
                    dst[:, g0 : g0 + gs, :],
                    src[g0 * 128 : (g0 + gs) * 128, :].rearrange(
                        "(j p) n -> p j n", p=128
                    ),
                )

            def slab_tile():
                return xs_pool.tile([128, NMI, KT, 128], FP8, name="xs")

            def slab_chunk(t, src, mg, mi, ci, nchunk):
                cs = KT // nchunk
                nc.sync.dma_start(
                    t[:, mi, ci * cs : (ci + 1) * cs, :],
                    src[mg, mi].rearrange("p (kt j) -> p kt j", kt=KT)[
                        :, ci * cs : (ci + 1) * cs, :
                    ],
                )

            pre = {}
            for mg in range(3):
                pre[mg] = (slab_tile(), slab_tile())

            # --- startup DMA stream, one queue (sync), hand-woven order ---
            # x-mg0 and x-mg1 chunks woven between the w_hi groups so the
            # PE's (main+xl)-interleaved consumption paces arrival; w_lo and
            # the bf16 correction operands follow (needed only at chain
            # ends, ~20us in); mg2's slab closes the stream.
            wsched = {
                0: [(0, 0, 0, 0), (0, 1, 0, 0)],   # x0h/x0l mi0 c0
                1: [(0, 0, 1, 0), (0, 1, 1, 0)],   # x0h/x0l mi1 c0
                2: [(0, 0, 0, 1), (0, 1, 0, 1)],   # x0h/x0l mi0 c1
                3: [(0, 0, 1, 1), (0, 1, 1, 1)],   # x0h/x0l mi1 c1
                4: [(1, 0, 0, 0), (1, 1, 0, 0)],   # x1h/x1l mi0 c0
                5: [(1, 0, 0, 1), (1, 1, 0, 1)],   # x1h/x1l mi0 c1
                6: [(1, 0, 1, 0), (1, 1, 1, 0)],   # x1h/x1l mi1 c0
                7: [(1, 0, 1, 1), (1, 1, 1, 1)],   # x1h/x1l mi1 c1
            }
            wg = 0
            for gi, gs in enumerate(WH_G):
                w_group(wht, wh, wg, gs)
                wg += gs
                for mgsel, part, mi, ci in wsched.get(gi, []):
                    slab_chunk(pre[mgsel][part], (xh, xl)[part],
                               mgsel, mi, ci, 2)
            wg = 0
            for gs in groups(WLT):
                w_group(wlt, wl, wg, gs)
                wg += gs
            nc.sync.dma_start(gxt[:], gx[:, :])
            nc.sync.dma_start(gwt[:], gw[:, :])
            for mi in range(NMI):
                slab_chunk(pre[2][0], xh, 2, mi, 0, 1)
                slab_chunk(pre[2][1], xl, 2, mi, 0, 1)

            def do_mm(psums_mi, mi, xt, wt, t, start, stop):
                lhsT = xt[:, mi, 2 * t : 2 * t + 2, :]
                for si, (n0, nw) in enumerate(NS):
                    nc.tensor.matmul(
                        psums_mi[si][:, :nw],
                        lhsT,
                        wt[:, 2 * t : 2 * t + 2, n0 : n0 + nw],
                        start=start,
                        stop=stop,
                        perf_mode=DR,
                    )

            def corr_mm(ps, si, mg, mi, stop=True):
                # rank-65 bf16 correction (+bias): [65,128]^T @ [65,nw]
                n0, nw = NS[si]
                m0 = mg * MGW + mi * 128
                nc.tensor.matmul(
                    ps[:, :nw],
                    gxt[:, m0 : m0 + 128],
                    gwt[:, n0 : n0 + nw],
                    start=False,
                    stop=stop,
                )

            def tail_mm(psums_mi, mi, xt, t, start=False, stop=False):
                # single w-corr / corr instructions per slice
                for si, (n0, nw) in enumerate(NS):
                    nc.tensor.matmul(
                        psums_mi[si][:, :nw],
                        xt[:, mi, 2 * t : 2 * t + 2, :],
                        wlt[:, 2 * t : 2 * t + 2, n0 : n0 + nw],
                        start=start,
                        stop=stop,
                        perf_mode=DR,
                    )

            def evict(psums_mi, mi, mg):
                ot = osb_pool.tile([128, Nc], F32, name="ot")
                for si, (n0, nw) in enumerate(NS):
                    nc.vector.tensor_copy(
                        ot[:, n0 : n0 + nw], psums_mi[si][:, :nw]
                    )
                m0 = mg * MGW + mi * 128
                nc.scalar.dma_start(out[m0 : m0 + 128, :], ot[:])

            # --- mg0 + head of mg1, interleaved per k-pair so consumption
            # follows the startup stream.  mg0: 6 psum banks; mg1.mi0's
            # first two slices use the 2 remaining banks.
            psums0 = [
                [psum_pool.tile([128, 512], F32, name="pt") for _ in NS]
                for _ in range(NMI)
            ]
            xsh0, xsl0 = pre[0]
            xsh1, xsl1 = pre[1]
            ps1_head = [psum_pool.tile([128, 512], F32, name="pt")
                        for _ in range(2)]

            def mg1_head_mm(xt, t, start):
                for si in (0, 1):
                    n0, nw = NS[si]
                    nc.tensor.matmul(
                        ps1_head[si][:, :nw],
                        xt[:, 0, 2*t:2*t+2, :],
                        wht[:, 2*t:2*t+2, n0:n0+nw],
                        start=start,
                        stop=False, perf_mode=DR)

            for t in range(NP):
                for mi in range(NMI):
                    do_mm(psums0[mi], mi, xsh0, wht, t,
                          start=(t == 0), stop=False)
                mg1_head_mm(xsh1, t, start=(t == 0))
                for mi in range(NMI):
                    do_mm(psums0[mi], mi, xsl0, wht, t,
                          start=False, stop=False)
                mg1_head_mm(xsl1, t, start=False)

            # mg0 chain tails: w-corr pairs + bf16 correction, then evict
            for mi in range(NMI):
                for t in range(WLP):
                    do_mm(psums0[mi], mi, xsh0, wlt, t,
                          start=False, stop=False)
                for si in range(len(NS)):
                    corr_mm(psums0[mi][si], si, 0, mi)
                evict(psums0[mi], mi, 0)

            # --- mg1 completion: mi0 s0/s1 need only their tails; the rest
            # are full chains on freshly freed banks.
            psums1 = [[None] * len(NS) for _ in range(NMI)]
            psums1[0][0], psums1[0][1] = ps1_head
            for mi in range(NMI):
                for si, (n0, nw) in enumerate(NS):
                    if mi == 0 and si < 2:
                        continue
                    ps = psum_pool.tile([128, 512], F32, name="pt")
                    psums1[mi][si] = ps
                    for t in range(NP):
                        nc.tensor.matmul(
                            ps[:, :nw],
                            xsh1[:, mi, 2*t:2*t+2, :],
                            wht[:, 2*t:2*t+2, n0:n0+nw],
                            start=(t == 0), stop=False, perf_mode=DR)
                        nc.tensor.matmul(
                            ps[:, :nw],
                            xsl1[:, mi, 2*t:2*t+2, :],
                            wht[:, 2*t:2*t+2, n0:n0+nw],
                            start=False, stop=False, perf_mode=DR)
            for mi in range(NMI):
                for t in range(WLP):
                    do_mm(psums1[mi], mi, xsh1, wlt, t,
                          start=False, stop=False)
                for si in range(len(NS)):
                    corr_mm(psums1[mi][si], si, 1, mi)
                evict(psums1[mi], mi, 1)

            # --- mg2..: steady state, sequential chains ---
            for mg in range(2, M // MGW):
                if mg in pre:
                    xsh_g, xsl_g = pre[mg]
                else:
                    xsh_g, xsl_g = slab_tile(), slab_tile()
                    for mi in range(NMI):
                        slab_chunk(xsh_g, xh, mg, mi, 0, 1)
                        slab_chunk(xsl_g, xl, mg, mi, 0, 1)
                for mi in range(NMI):
                    psums_mi = [
                        psum_pool.tile([128, 512], F32, name="pt")
                        for _ in NS
                    ]
                    for t in range(NP):
                        do_mm(psums_mi, mi, xsh_g, wht, t,
                              start=(t == 0), stop=False)
                        do_mm(psums_mi, mi, xsl_g, wht, t,
                              start=False, stop=False)
                    for t in range(WLP):
                        do_mm(psums_mi, mi, xsh_g, wlt, t,
                              start=False, stop=False)
                    for si in range(len(NS)):
                        corr_mm(psums_mi[si], si, mg, mi)
                    evict(psums_mi, mi, mg)

    nc.compile()
    return nc


def _e4m3_split(a):
    """Return (hi, lo) e4m3 arrays with hi + lo ~= a (both as fp8 views)."""
    f8 = ml_dtypes.float8_e4m3
    hi = a.astype(f8)
    lo = (a - hi.astype(np.float32)).astype(f8)
    return hi, lo


def make_in_maps(input_np, q_weights, q_scales, q_zp, bias, ncores=NCORES):
    """Host-side prep: recentered dequant / fp8-split / transpose / shard."""
    n = q_weights.shape[0]
    nc_sh = n // ncores
    m, k = input_np.shape
    kt = k // 128

    x32 = input_np.astype(np.float32)
    xhi, xlo = _e4m3_split(x32)

    nmi = MGW // 128

    def xfmt(a):
        # [mg, mi, p, kt, j] = a[mg*MGW+mi*128+j, kt*128+p]
        return np.ascontiguousarray(
            a.T.reshape(kt, 128, m // MGW, nmi, 128).transpose(2, 3, 1, 0, 4)
        ).reshape(m // MGW, nmi, 128, kt * 128)

    xh = xfmt(xhi)
    xl = xfmt(xlo)

    # correction lhsT: 64 block sums of x + ones row, [65, M] bf16
    bf = ml_dtypes.bfloat16
    xs = x32.reshape(m, NB, BLOCK).sum(axis=2)          # [M, 64]
    gx = np.empty((65, m), dtype=bf)
    gx[:64] = xs.T.astype(bf)
    gx[64] = np.float32(1.0)

    maps = []
    for i in range(ncores):
        sl = slice(i * nc_sh, (i + 1) * nc_sh)
        q = q_weights[sl].astype(np.float32).reshape(nc_sh, NB, BLOCK)
        s = q_scales[sl].astype(np.float32)             # [nc, 64]
        zp = q_zp[sl].astype(np.float32)                # [nc, 64]
        c = q.mean(axis=2)                              # [nc, 64]
        wp = ((q - c[:, :, None]) * s[:, :, None]).reshape(nc_sh, k)
        whi = wp.astype(ml_dtypes.float8_e4m3)
        wl_rows = WLT * 128
        wlo = (wp[:, :wl_rows]
               - whi[:, :wl_rows].astype(np.float32)).astype(
                   ml_dtypes.float8_e4m3)
        gvals = s * (c - zp)                            # [nc, 64]
        gw = np.empty((65, nc_sh), dtype=bf)
        gw[:64] = gvals.T.astype(bf)
        gw[64] = bias[sl].astype(bf)
        maps.append(
            {
                "xh": xh,
                "xl": xl,
                "wh": np.ascontiguousarray(whi.T),
                "wl": np.ascontiguousarray(wlo.T),
                "gx": gx,
                "gw": gw,
            }
        )
    return maps


_PROGRAM = None


def _get_program():
    global _PROGRAM
    if _PROGRAM is None:
        _PROGRAM = build_program(M, K, NC_SHARD, MGW)
    return _PROGRAM


def kernel(input, q_weights, q_scales, q_zp, bias):
    """Full unsharded inputs -> full [M, N] float32 output."""
    input = np.asarray(input, dtype=np.float32)
    q_weights = np.asarray(q_weights, dtype=np.int32)
    q_scales = np.asarray(q_scales, dtype=np.float32)
    q_zp = np.asarray(q_zp, dtype=np.int32)
    bias = np.asarray(bias, dtype=np.float32)

    nc = _get_program()
    maps = make_in_maps(input, q_weights, q_scales, q_zp, bias)
    res = run_bass_kernel_spmd(nc, maps, core_ids=list(range(NCORES)))
    return np.concatenate(
        [res.results[i]["out"] for i in range(NCORES)], axis=1
    )
